# revision 20
# baseline (speedup 1.0000x reference)
"""Trainium2 Bass kernel for a GPT-2 style transformer block (pre-LN, no mask).

Reference shapes: x [B=2, T=2048, C=1024], H=16 heads, MLP hidden 4C=4096.

Sharding (8 NeuronCores): data-parallel over B (cores 0-3 -> batch 0,
cores 4-7 -> batch 1); within each 4-core group the 2048 query rows are
split 512 per core. Every core redundantly computes K and V for its full
batch from a replicated (rotated) copy of x, so no collectives are needed.

v2: fp8 (e4m3) DoubleRow matmuls for QKV / attn-proj / MLP (2 fp8 MACs
per PE cell per cycle, 256-deep contraction), fp8 softmax probabilities
feeding a DoubleRow P@V, and the exp stream split between the scalar
engine (hardware Exp) and the vector engine (Schraudolph int-bitcast
fast-exp) so the attention phase is no longer scalar-bound. Scores stay
bf16. Residual stream stays f32.
"""

import numpy as np
import ml_dtypes

import concourse.bass as bass
import concourse.bacc as bacc
import concourse.tile as tile
from concourse import mybir
from concourse.bass import ts, ds
from concourse.bass_utils import run_bass_kernel_spmd

f32 = mybir.dt.float32
bf16 = mybir.dt.bfloat16
fp8 = mybir.dt.float8e4
i32 = mybir.dt.int32
AF = mybir.ActivationFunctionType
OP = mybir.AluOpType
PM = mybir.MatmulPerfMode

B, T, C, H = 2, 2048, 1024, 16
DH = C // H          # 64
F = 4 * C            # 4096
NCORES = 8
GROUP = 4            # cores per batch
TQ = T // GROUP      # 512 query rows per core
NT = T // 128        # 16 token tiles
CCH = C // 128       # 8 contraction chunks over C
PAIRS = H // 2       # 8 head pairs
FT = F // 128        # 32 hidden tiles
QT = TQ // 128       # 4 own-row tiles

f16 = mybir.dt.float16

# Schraudolph fast-exp constants: exp(x) ~= bitcast_f32(int(EA*x + EB))
EA = 12102203.161561485   # 2^23 / ln 2
EB = 1064866805.0         # 127*2^23 - 486411 (min-max relative error)
# cidx values whose exp runs on the vector engine (rest on scalar engine)
DVE_EXP = frozenset((1, 4, 7, 10, 13))

_CACHED = {}


def _bcast(ap, parts=128):
    """DRAM AP for a 1-D tensor broadcast across `parts` partitions."""
    return bass.AP(tensor=ap.tensor, offset=ap.offset, ap=[[0, parts]] + list(ap.ap))


def _build_program(trivial_ln1, trivial_ln2, trivial_b):
    nc = bacc.Bacc("TRN2", target_bir_lowering=False, debug=False,
                   num_devices=NCORES)

    # host-transposed rotated x, chunk-major: [c%128, chunk, tile, token%128]
    xfT = nc.dram_tensor("xfT", [128, CCH, NT, 128], bf16, kind="ExternalInput")
    xq = nc.dram_tensor("xq", [TQ, C], f32, kind="ExternalInput")
    # pre-tiled weights: [128 (c within chunk), CCH, out-features] fp8
    wq = nc.dram_tensor("wq", [128, CCH, C], fp8, kind="ExternalInput")
    wk = nc.dram_tensor("wk", [128, CCH, C], fp8, kind="ExternalInput")
    wv = nc.dram_tensor("wv", [128, CCH, C], fp8, kind="ExternalInput")
    bqv = nc.dram_tensor("bq", [128, PAIRS], f32, kind="ExternalInput")
    bkv = nc.dram_tensor("bk", [128, PAIRS], f32, kind="ExternalInput")
    bvv = nc.dram_tensor("bv", [C], f32, kind="ExternalInput")
    ln1w = nc.dram_tensor("ln1w", [C], f32, kind="ExternalInput")
    ln1b = nc.dram_tensor("ln1b", [C], f32, kind="ExternalInput")
    ln2w = nc.dram_tensor("ln2w", [C], f32, kind="ExternalInput")
    ln2b = nc.dram_tensor("ln2b", [C], f32, kind="ExternalInput")
    wp = nc.dram_tensor("wp", [128, CCH, C], fp8, kind="ExternalInput")
    bp = nc.dram_tensor("bp", [C], f32, kind="ExternalInput")
    # wf pre-tiled per f'-tile, split: chunks 0..3 fp8 (DoubleRow), 4..7 f16
    wf8 = nc.dram_tensor("wf8", [FT, 128, CCH // 2, 128], fp8, kind="ExternalInput")
    wff = nc.dram_tensor("wff", [FT, 128, CCH // 2, 128], f16, kind="ExternalInput")
    bf_ = nc.dram_tensor("bf", [128, FT], f32, kind="ExternalInput")
    wm = nc.dram_tensor("wm", [F, C], f16, kind="ExternalInput")
    bm = nc.dram_tensor("bm", [C], f32, kind="ExternalInput")
    out = nc.dram_tensor("out", [TQ, C], f32, kind="ExternalOutput")

    with tile.TileContext(nc) as tc:
        _emit(nc, tc, trivial_ln1, trivial_ln2, trivial_b,
              xfT, xq, wq, wk, wv, bqv, bkv, bvv, ln1w, ln1b, ln2w, ln2b,
              wp, bp, wf8, wff, bf_, wm, bm, out)
    nc.compile()
    return nc


def _colmajor(ap):
    """DRAM AP for a [C] vector viewed as [128, CCH] (partition = c % 128)."""
    return bass.AP(tensor=ap.tensor, offset=ap.offset, ap=[[1, 128], [128, CCH]])


def _emit(nc, tc, trivial_ln1, trivial_ln2, trivial_b,
          xfT, xq, wq, wk, wv, bqv, bkv, bvv, ln1w, ln1b, ln2w, ln2b,
          wp, bp, wf8, wff, bf_, wm, bm, out):
    from contextlib import ExitStack

    with ExitStack() as st:
        persist = st.enter_context(tc.tile_pool(name="persist", bufs=1))
        stat = st.enter_context(tc.tile_pool(name="stat", bufs=4))
        stream = st.enter_context(tc.tile_pool(name="stream", bufs=5))

        eps_t = persist.tile([128, 1], f32)
        nc.vector.memset(eps_t, 1e-5)

        def layer_norm(x_t, w_bc, b_bc, out_ap, trivial):
            """x_t [128, C] f32 -> out_ap [128, C] bf16 (normalized + affine)."""
            stats = stat.tile([128, 2, nc.vector.BN_STATS_DIM], f32, name="stats", bufs=6)
            nc.vector.bn_stats(out=stats[:, 0, :], in_=x_t[:, 0:512])
            nc.vector.bn_stats(out=stats[:, 1, :], in_=x_t[:, 512:1024])
            mv = stat.tile([128, nc.vector.BN_AGGR_DIM], f32, name="mv", bufs=6)
            nc.vector.bn_aggr(out=mv, in_=stats)
            rstd = stat.tile([128, 1], f32, name="rstd", bufs=6)
            nc.scalar.activation(rstd, mv[:, 1:2], AF.Sqrt, bias=eps_t)
            nc.vector.reciprocal(rstd, rstd)
            if trivial:
                nc.vector.tensor_scalar(out=out_ap, in0=x_t, scalar1=mv[:, 0:1],
                                        scalar2=rstd, op0=OP.subtract, op1=OP.mult)
            else:
                t1 = stat.tile([128, C], f32, name="t1", tag="ln_t1")
                nc.vector.tensor_scalar(out=t1, in0=x_t, scalar1=mv[:, 0:1],
                                        scalar2=rstd, op0=OP.subtract, op1=OP.mult)
                nc.vector.tensor_mul(t1, t1, w_bc)
                nc.vector.tensor_add(out_ap, t1, b_bc)

        # ---------------- pools (stack discipline per side) ----------------
        stA = st.enter_context(ExitStack())
        pA = stA.enter_context(tc.tile_pool(name="pA", bufs=1, side="left"))
        pR = st.enter_context(tc.tile_pool(name="pR", bufs=1, side="right"))
        stB = st.enter_context(ExitStack())
        pB = stB.enter_context(tc.tile_pool(name="pB", bufs=1, side="right"))

        wv_sb = pB.tile([128, CCH, C], fp8)
        nc.sync.dma_start(out=wv_sb, in_=wv.ap())
        if not trivial_b:
            bv_bc = pA.tile([128, C], f32)
            nc.sync.dma_start(out=bv_bc, in_=_bcast(bvv.ap()))
        else:
            bv_bc = None
        if not trivial_ln1:
            ln1w_cb = pA.tile([128, CCH], f32)
            nc.sync.dma_start(out=ln1w_cb, in_=_colmajor(ln1w.ap()))
            ln1b_cb = pA.tile([128, CCH], f32)
            nc.sync.dma_start(out=ln1b_cb, in_=_colmajor(ln1b.ap()))
        else:
            ln1w_cb = ln1b_cb = None

        ones_bf = persist.tile([128, 1], bf16)
        nc.vector.memset(ones_bf, 1.0)

        # chunk-major transposed activations (fp8): [c%128, chunk, tile, token]
        hT8 = pA.tile([128, CCH, NT, 128], fp8)
        # V (fp8): [key%128, keytile pair, parity, head, DH | ones]
        v8 = pR.tile([128, NT // 2, 2, H, DH + 1], fp8)
        ynT = pR.tile([128, PAIRS, TQ], fp8)
        nc.vector.memset(v8[:, :, :, :, DH:DH + 1], 1.0)

        # ---- transposed LN1 (no xbar transpose): stats via PE ones-sums,
        # broadcast via gpsimd, normalization applied feature-major on DVE;
        # V matmuls interleave per 4-tile group ----
        with nc.named_scope("p1_ln_v"):
            with tc.tile_pool(name="v_ps", bufs=4, space="PSUM") as v_ps, \
                 tc.tile_pool(name="st_ps", bufs=2, space="PSUM") as st_ps:
                for g in range(NT // 4):
                    xg = stream.tile([128, CCH, 512], bf16, name="xg",
                                     tag="xg", bufs=2)
                    nc.sync.dma_start(out=xg, in_=xfT.ap()[:, :, 4 * g:4 * g + 4, :])
                    ps_sum = st_ps.tile([1, 512], f32, name="ps_sum", tag="ps_sum")
                    ps_sq = st_ps.tile([1, 512], f32, name="ps_sq", tag="ps_sq")
                    for c in range(CCH):
                        nc.tensor.matmul(ps_sum, ones_bf, xg[:, c, :],
                                         start=(c == 0), stop=(c == CCH - 1))
                        sq = stream.tile([128, 512], bf16, name="sq", tag="sq",
                                         bufs=3)
                        nc.vector.tensor_mul(sq, xg[:, c, :], xg[:, c, :])
                        nc.tensor.matmul(ps_sq, ones_bf, sq,
                                         start=(c == 0), stop=(c == CCH - 1))
                    mrow = stat.tile([1, 4, 512], f32, name="mrow", tag="mrow",
                                     bufs=2)
                    nc.vector.tensor_scalar_mul(mrow[:, 0, :], ps_sum, 1.0 / C)
                    nc.vector.tensor_scalar_mul(mrow[:, 1, :], ps_sq, 1.0 / C)
                    nc.vector.tensor_mul(mrow[:, 2, :], mrow[:, 0, :], mrow[:, 0, :])
                    nc.vector.tensor_sub(mrow[:, 1, :], mrow[:, 1, :], mrow[:, 2, :])
                    nc.scalar.activation(mrow[:, 2, :], mrow[:, 1, :], AF.Sqrt,
                                         bias=eps_t[0:1, :])
                    nc.vector.reciprocal(mrow[:, 3, :], mrow[:, 2, :])
                    bc = stream.tile([128, 2, 512], f32, name="bc", tag="bc",
                                     bufs=2)
                    nc.gpsimd.partition_broadcast(bc[:, 0, :], mrow[:, 0, :])
                    nc.gpsimd.partition_broadcast(bc[:, 1, :], mrow[:, 3, :])
                    for c in range(CCH):
                        dst = hT8[:, c, 4 * g:4 * g + 4, :]
                        tmp = stream.tile([128, 512], bf16, name="tmp", tag="sq",
                                          bufs=3)
                        nc.vector.tensor_sub(tmp, xg[:, c, :], bc[:, 0, :])
                        if trivial_ln1:
                            nc.vector.tensor_mul(dst, tmp, bc[:, 1, :])
                        else:
                            nc.vector.tensor_mul(tmp, tmp, bc[:, 1, :])
                            nc.vector.tensor_scalar(out=dst, in0=tmp,
                                                    scalar1=ln1w_cb[:, c:c + 1],
                                                    scalar2=ln1b_cb[:, c:c + 1],
                                                    op0=OP.mult, op1=OP.add)
                    if g == 1:
                        wk_sb = pA.tile([128, CCH, C], fp8)
                        nc.sync.dma_start(out=wk_sb, in_=wk.ap())
                    if g == 2:
                        wq_sb = pA.tile([128, CCH, C], fp8)
                        nc.sync.dma_start(out=wq_sb, in_=wq.ap())
                        bq_sb = pA.tile([128, PAIRS], f32)
                        nc.sync.dma_start(out=bq_sb, in_=bqv.ap())
                        bk_sb = pA.tile([128, PAIRS], f32)
                        nc.sync.dma_start(out=bk_sb, in_=bkv.ap())
                    for it in range(4):
                        i = 4 * g + it
                        pss = [v_ps.tile([128, 512], f32, name=f"ps_v{n}",
                                         tag="ps_v") for n in range(2)]
                        for c2 in range(CCH // 2):
                            for n in range(C // 512):
                                nc.tensor.matmul(
                                    pss[n], hT8[:, 2 * c2:2 * c2 + 2, i, :],
                                    wv_sb[:, 2 * c2:2 * c2 + 2, ds(512 * n, 512)],
                                    start=(c2 == 0), stop=(c2 == CCH // 2 - 1),
                                    perf_mode=PM.DoubleRow)
                        e_, par = i // 2, i % 2
                        for n in range(C // 512):
                            dst = v8[:, e_, par, 8 * n:8 * n + 8, 0:DH]
                            if trivial_b:
                                # split psum evacuation between scalar and vector
                                if n == 0:
                                    nc.scalar.activation(dst, pss[n], AF.Identity)
                                else:
                                    nc.vector.tensor_copy(dst, pss[n])
                            else:
                                nc.vector.tensor_add(dst, pss[n],
                                                     bv_bc[:, ds(512 * n, 512)])
        stB.close()

        # wp prefetch during attention (DMA is idle there)
        wp_sb = pR.tile([128, CCH, C], fp8)
        nc.sync.dma_start(out=wp_sb, in_=wp.ap())
        if not trivial_ln2:
            ln2w_bc = pR.tile([128, C], f32)
            nc.sync.dma_start(out=ln2w_bc, in_=_bcast(ln2w.ap()))
            ln2b_bc = pR.tile([128, C], f32)
            nc.sync.dma_start(out=ln2b_bc, in_=_bcast(ln2b.ap()))
        else:
            ln2w_bc = ln2b_bc = None
        bp_bc = pR.tile([128, C], f32)
        nc.sync.dma_start(out=bp_bc, in_=_bcast(bp.ap()))

        # ---- per-pair K^T/Q^T + attention, interleaved ----
        scale = 1.0 / float(np.sqrt(DH))
        with nc.named_scope("p2_attn"), \
             tc.tile_pool(name="kq_ps", bufs=2, space="PSUM") as kq_ps, \
             tc.tile_pool(name="s_ps", bufs=2, space="PSUM") as s_ps, \
             tc.tile_pool(name="y_ps", bufs=1, space="PSUM") as y_ps, \
             tc.tile_pool(name="kq_sb", bufs=2) as kq_sb, \
             tc.tile_pool(name="att_sb", bufs=3) as att_sb:
            for j in range(PAIRS):
                kT_j = kq_sb.tile([128, T], bf16, name="kT_j", tag="kT_j")
                for ng in range(T // 1024):
                    psn = [kq_ps.tile([128, 512], f32, name=f"ps_k{v}",
                                      tag="ps_kq") for v in range(2)]
                    for c2 in range(CCH // 2):
                        for v in range(2):
                            n = 2 * ng + v
                            nc.tensor.matmul(psn[v],
                                             wk_sb[:, 2 * c2:2 * c2 + 2, ts(j, 128)],
                                             hT8[:, 2 * c2:2 * c2 + 2, 4 * n:4 * n + 4, :],
                                             start=(c2 == 0), stop=(c2 == CCH // 2 - 1),
                                             perf_mode=PM.DoubleRow)
                    for v in range(2):
                        n = 2 * ng + v
                        nc.vector.tensor_scalar(out=kT_j[:, ds(512 * n, 512)],
                                                in0=psn[v], scalar1=bk_sb[:, j:j + 1],
                                                scalar2=None, op0=OP.add)
                qT_j = kq_sb.tile([128, TQ], bf16, name="qT_j", tag="qT_j")
                ps = kq_ps.tile([128, 512], f32, name="ps_q", tag="ps_kq")
                for c2 in range(CCH // 2):
                    nc.tensor.matmul(ps, wq_sb[:, 2 * c2:2 * c2 + 2, ts(j, 128)],
                                     hT8[:, 2 * c2:2 * c2 + 2, 0:QT, :],
                                     start=(c2 == 0), stop=(c2 == CCH // 2 - 1),
                                     perf_mode=PM.DoubleRow)
                nc.vector.tensor_scalar(out=qT_j, in0=ps,
                                        scalar1=bq_sb[:, j:j + 1],
                                        scalar2=None, op0=OP.add)

                ps_y1 = y_ps.tile([DH + 1, 512], f32, name="ps_y1", tag="ps_y1")
                ps_y2 = y_ps.tile([DH + 1, 512], f32, name="ps_y2", tag="ps_y2")
                # software-pipelined: PV(cidx-1) is emitted after S(cidx) so the
                # PE never round-trips with the exp engines within a tile
                pT_q = []
                for cidx in range(NT):
                    ps_s = s_ps.tile([128, 1024], f32, name="ps_s", tag="ps_s")
                    nc.tensor.matmul(ps_s[:, 0:512],
                                     kT_j[0:64, ts(cidx, 128)],
                                     qT_j[0:64, :], start=True, stop=True)
                    nc.tensor.matmul(ps_s[:, 512:1024],
                                     kT_j[64:128, ts(cidx, 128)],
                                     qT_j[64:128, :], start=True, stop=True,
                                     tile_position=(64, 0))
                    pT = att_sb.tile([128, 2, TQ], bf16, name="pT", tag="pT")
                    if cidx in DVE_EXP:
                        t32 = att_sb.tile([128, 1024], i32, name="t32",
                                          tag="t32", bufs=2)
                        nc.vector.tensor_scalar(out=t32, in0=ps_s,
                                                scalar1=EA * scale, scalar2=EB,
                                                op0=OP.mult, op1=OP.add)
                        nc.vector.tensor_copy(pT, t32[:].bitcast(f32))
                    else:
                        nc.scalar.activation(pT, ps_s, AF.Exp, scale=scale)
                    pT_q.append(pT)
                    if cidx >= 1:
                        pv = cidx - 1
                        for u in range(2):
                            nc.tensor.matmul(ps_y1 if u == 0 else ps_y2,
                                             v8[:, pv // 2, pv % 2, 2 * j + u, :],
                                             pT_q[pv][:, u, :],
                                             start=(pv == 0), stop=False)
                pv = NT - 1
                for u in range(2):
                    nc.tensor.matmul(ps_y1 if u == 0 else ps_y2,
                                     v8[:, pv // 2, pv % 2, 2 * j + u, :],
                                     pT_q[pv][:, u, :],
                                     start=False, stop=True)
                for u, ps_y in ((0, ps_y1), (1, ps_y2)):
                    # copy Y and the sums row out of PSUM right away so the
                    # accumulator banks free up for the next pair; the sums
                    # staging copy also moves them to SBUF partition 0
                    # (custom-DVE ops mis-read PSUM at a partition offset)
                    ycp = att_sb.tile([64, 512], f32, name="ycp", tag="ycp")
                    nc.vector.tensor_copy(ycp, ps_y[0:DH, :])
                    rs0 = att_sb.tile([1, 512], f32, name="rs0", tag="rs0")
                    nc.vector.tensor_copy(rs0, ps_y[DH:DH + 1, :])
                    rs = att_sb.tile([1, 512], f32, name="rs", tag="rs")
                    nc.vector.reciprocal_approx_fast(rs, rs0)
                    bc = att_sb.tile([64, 512], f32, name="bc", tag="bc")
                    nc.gpsimd.partition_broadcast(bc, rs)
                    nc.vector.tensor_mul(ynT[64 * u:64 * u + 64, j, :],
                                         ycp, bc)
        stA.close()

        # ---- attn projection + residual + LN2 + h2^T ----
        pD = st.enter_context(tc.tile_pool(name="pD", bufs=1, side="left"))
        x2 = pD.tile([128, QT, C], f32)
        h2Tb = pD.tile([128, CCH, QT, 128], f16)
        h2T8 = pD.tile([128, CCH // 2, QT, 128], fp8)
        bfc_sb = pD.tile([128, FT], f32)
        nc.sync.dma_start(out=bfc_sb, in_=bf_.ap())
        bm_bc = pD.tile([128, C], f32)
        nc.sync.dma_start(out=bm_bc, in_=_bcast(bm.ap()))

        with nc.named_scope("p3_proj_ln2"):
            with tc.tile_pool(name="ap_ps", bufs=2, space="PSUM") as ap_ps:
                for i in range(QT):
                    xb_t = stream.tile([128, C], f32, name="xb_t", tag="x_t")
                    nc.sync.dma_start(out=xb_t, in_=xq.ap()[ts(i, 128), :])
                    nc.vector.tensor_add(xb_t, xb_t, bp_bc)
                    for n in range(C // 512):
                        ps = ap_ps.tile([128, 512], f32, name="ps_a", tag="ps_a")
                        for a in range(PAIRS // 2):
                            nc.tensor.matmul(ps, ynT[:, 2 * a:2 * a + 2, ts(i, 128)],
                                             wp_sb[:, 2 * a:2 * a + 2, ds(512 * n, 512)],
                                             start=(a == 0), stop=(a == PAIRS // 2 - 1),
                                             perf_mode=PM.DoubleRow)
                        nc.vector.tensor_add(x2[:, i, ds(512 * n, 512)], ps,
                                             xb_t[:, ds(512 * n, 512)])
                    h2_t = stream.tile([128, C], f16, name="h2_t", tag="h2_t", bufs=5)
                    layer_norm(x2[:, i, :], ln2w_bc, ln2b_bc, h2_t, trivial_ln2)
                    nc.sync.dma_start_transpose(h2Tb[:, :, i, :], h2_t[:])
                    nc.scalar.activation(h2T8[:, :, i, :], h2Tb[:, 0:CCH // 2, i, :],
                                         AF.Identity)

        # ---- MLP ----
        # fold the mlp_proj bias into the residual copy while fc runs (DVE idle)
        for i in range(QT):
            nc.vector.tensor_add(x2[:, i, :], x2[:, i, :], bm_bc)
        gTf = pD.tile([128, FT, TQ], f16)
        with nc.named_scope("p4_fc"):
            with tc.tile_pool(name="fc_ps", bufs=4, space="PSUM") as fc_ps, \
                 tc.tile_pool(name="wf_sb", bufs=4) as wf_pool:
                for t in range(FT):
                    wf8_t = wf_pool.tile([128, CCH // 2, 128], fp8, name="wf8_t",
                                         tag="wf8_t")
                    nc.sync.dma_start(out=wf8_t, in_=wf8.ap()[t])
                    wff_t = wf_pool.tile([128, CCH // 2, 128], f16, name="wff_t",
                                         tag="wff_t")
                    nc.sync.dma_start(out=wff_t, in_=wff.ap()[t])
                    ps = fc_ps.tile([128, 512], f32, name="ps_f", tag="ps_f")
                    for c2 in range(2):
                        nc.tensor.matmul(ps, wf8_t[:, 2 * c2:2 * c2 + 2, :],
                                         h2T8[:, 2 * c2:2 * c2 + 2, 0:QT, :],
                                         start=(c2 == 0), stop=False,
                                         perf_mode=PM.DoubleRow)
                    for cx in range(CCH // 2):
                        nc.tensor.matmul(ps, wff_t[:, cx, :],
                                         h2Tb[:, CCH // 2 + cx, 0:QT, :],
                                         start=False, stop=(cx == CCH // 2 - 1))
                    nc.scalar.activation(gTf[:, t, :], ps, AF.Gelu_apprx_tanh,
                                         bias=bfc_sb[:, t:t + 1], scale=1.0)

        with nc.named_scope("p5_mlp_out"):
            with tc.tile_pool(name="m_ps", bufs=1, space="PSUM") as m_ps, \
                 tc.tile_pool(name="wm_sb", bufs=5) as wm_pool, \
                 tc.tile_pool(name="out_sb", bufs=2) as out_pool:
                ps_m = [m_ps.tile([128, 512], f32, name=f"ps_m{k}", tag=f"ps_m{k}")
                        for k in range(8)]
                for t in range(FT):
                    last = t == FT - 1
                    wm_t = wm_pool.tile([128, C], f16, name="wm_t", tag="wm_t")
                    nc.sync.dma_start(out=wm_t, in_=wm.ap()[ts(t, 128), :])
                    for i in range(QT):
                        for n in range(C // 512):
                            nc.tensor.matmul(ps_m[i * 2 + n],
                                             gTf[:, t, ts(i, 128)],
                                             wm_t[:, ds(512 * n, 512)],
                                             start=(t == 0), stop=last)
                        if last:
                            # drain this i's accumulators immediately so the
                            # final adds + output DMA overlap the remaining MMs
                            out_t = out_pool.tile([128, C], f32, name="out_t",
                                                  tag="out_t")
                            for n in range(C // 512):
                                nc.vector.tensor_add(out_t[:, ds(512 * n, 512)],
                                                     ps_m[i * 2 + n],
                                                     x2[:, i, ds(512 * n, 512)])
                                nc.sync.dma_start(
                                    out=out.ap()[ts(i, 128), ds(512 * n, 512)],
                                    in_=out_t[:, ds(512 * n, 512)])


def _get_program(trivial_ln1, trivial_ln2, trivial_b):
    key = (trivial_ln1, trivial_ln2, trivial_b)
    if key not in _CACHED:
        _CACHED[key] = _build_program(trivial_ln1, trivial_ln2, trivial_b)
    return _CACHED[key]


def _fp8(a):
    return np.ascontiguousarray(np.asarray(a, np.float32)
                                .clip(-240, 240).astype(ml_dtypes.float8_e4m3))


def _tile_proj_weight(w):
    # [C, N] f32 -> [128, CCH, N] fp8 with partition = c % 128, chunk = c // 128
    w = np.asarray(w, np.float32).reshape(CCH, 128, -1)
    return _fp8(w.transpose(1, 0, 2))


def _prep_in_maps(inputs):
    fl = lambda a: np.ascontiguousarray(np.asarray(a, np.float32))
    x = fl(inputs["x"])
    attn_w = fl(inputs["attn_w"])
    attn_b = fl(inputs["attn_b"])
    wf_full = fl(inputs["fc_w"])  # [C, F]
    # wf tiled: [FT, 128(c), CCH, 128(f')]; chunks 0..3 fp8, 4..7 f16
    wf_t = np.ascontiguousarray(
        wf_full.reshape(CCH, 128, FT, 128).transpose(2, 1, 0, 3))
    wf8_t = _fp8(wf_t[:, :, 0:CCH // 2, :])
    wff_t = np.ascontiguousarray(wf_t[:, :, CCH // 2:, :].astype(np.float16))
    wm_t = np.ascontiguousarray(fl(inputs["mlp_proj_w"]).astype(np.float16))
    pb = lambda b: np.ascontiguousarray(
        np.asarray(b, np.float32).reshape(-1, 128).T)  # [128, tiles]
    shared = {
        "wq": _tile_proj_weight(attn_w[:, 0:C]),
        "wk": _tile_proj_weight(attn_w[:, C:2 * C]),
        "wv": _tile_proj_weight(attn_w[:, 2 * C:3 * C]),
        "bq": pb(attn_b[0:C]), "bk": pb(attn_b[C:2 * C]),
        "bv": fl(attn_b[2 * C:3 * C]),
        "ln1w": fl(inputs["ln1_w"]), "ln1b": fl(inputs["ln1_b"]),
        "ln2w": fl(inputs["ln2_w"]), "ln2b": fl(inputs["ln2_b"]),
        "wp": _tile_proj_weight(inputs["attn_proj_w"]),
        "bp": fl(inputs["attn_proj_b"]),
        "wf8": wf8_t, "wff": wff_t, "bf": pb(inputs["fc_b"]),
        "wm": wm_t,
        "bm": fl(inputs["mlp_proj_b"]),
    }
    in_maps = []
    for core in range(NCORES):
        b, r = core // GROUP, core % GROUP
        xb = np.roll(x[b], -TQ * r, axis=0)
        xbT = (xb.reshape(NT, 128, CCH, 128).transpose(3, 2, 0, 1)
               .astype(ml_dtypes.bfloat16))
        in_maps.append({
            "xfT": np.ascontiguousarray(xbT),
            "xq": np.ascontiguousarray(xb[0:TQ]),
            **shared,
        })
    return in_maps


def run(inputs, trace=False):
    trivial_ln1 = bool(np.all(np.asarray(inputs["ln1_w"]) == 1.0)
                       and np.all(np.asarray(inputs["ln1_b"]) == 0.0))
    trivial_ln2 = bool(np.all(np.asarray(inputs["ln2_w"]) == 1.0)
                       and np.all(np.asarray(inputs["ln2_b"]) == 0.0))
    trivial_b = bool(np.all(np.asarray(inputs["attn_b"]) == 0.0))
    nc = _get_program(trivial_ln1, trivial_ln2, trivial_b)
    in_maps = _prep_in_maps(inputs)
    res = run_bass_kernel_spmd(nc, in_maps, core_ids=list(range(NCORES)),
                               trace=trace)
    out = np.empty((B, T, C), np.float32)
    for core in range(NCORES):
        b, r = core // GROUP, core % GROUP
        out[b, TQ * r:TQ * (r + 1)] = res.results[core]["out"]
    return out, res


def kernel(**inputs):
    out, _ = run(inputs, trace=False)
    return out
